# revision 44
# baseline (speedup 1.0000x reference)
"""Transformer-XL block (relative-position attention + MLP) on 8 TRN2 NeuronCores.

Sharding (unchanged from v1): core c handles batch b = c//2, query rows
[lo, lo+256), lo = 256*(c%2). Data-parallel; no collectives.

v2 performance structure:
  - q/k/r/v projections run in fp8e4 DoubleRow (256-deep contraction per pass):
    weights host-scaled x64 into fp8 range, descaled 1/64 at PSUM evict.
    Attention-path fp8 error is diluted by the diffuse softmax + residual + LN
    (host-emulated rel err 1.9e-3 vs the 2e-2 gate).
  - Attention is software-pipelined with a 2/3-step lag: per step s the PE
    stream carries BD(s) | AC(s-2) | ctx(s-3), so the BD -> DRAM -> shifted
    transpose-read roundtrip of head s never stalls the PE.
  - Softmax row-sums ride the ctx matmul as a 65th ones-column of v (zero extra
    PE cycles); normalization uses a [1,256] reciprocal + gpsimd
    partition_broadcast instead of the 8-matmul ones trick + [128,256]
    reciprocal.
  - Halo BD rows (q row lo+256) are computed head-pair-batched (blockdiag
    lhsT [128,2]) and written to the y buffers at phase A2, off phase B.
  - Startup: phase-A DMAs spread across 5 rings critical-path-first (fp8 halves
    the bytes), warm-up matmuls wake the HAM clock gate, the per-core shift
    register load is deferred to A2 (off the preamble critical path).
  - MLP: W2 runs t-outer so LN2 + store of the first row-half overlap the
    second half's matmuls; first W1 chunks prefetch during attention.

PSUM discipline: phases own disjoint pool sets; every matmul output tile is
<= 1 bank except k/r/v/Wo/W2 accumulators (2 banks, split 512-wide per instr).
"""

import numpy as np

import concourse.bass as bass
import concourse.tile as tile
from concourse.tile import add_dep_helper
from concourse import bacc, mybir
from concourse.bass_utils import run_bass_kernel_spmd
from concourse.masks import make_identity

F32 = mybir.dt.float32
BF16 = mybir.dt.bfloat16
FP8 = mybir.dt.float8e4
U32 = mybir.dt.uint32
NP_BF16 = mybir.dt.np(BF16)
NP_FP8 = mybir.dt.np(FP8)

B, Q, M, D, H, DH = 4, 512, 512, 1024, 16, 64
KL = M + Q            # 1024
QL = 256              # local q rows per core
HK = H * DH           # 1024
F = 4 * D             # 4096
P = 128
NCORES = 8
YW = KL + 1           # 1025, padded y row width
YROWS = QL + 1        # 257
XQW = 272             # xqT free width, padded 257 -> %16 for DoubleRow strides
VW = DH + 1           # 65: per-head v block width (64 ctx cols + ones col)
EPS = 1e-5
WSCALE = 64.0         # fp8 weight scale
DBL = mybir.MatmulPerfMode.DoubleRow

_cache = {}


def mm_acc(nc, psum, lhsT, rhs, first, last, nmax=512):
    """bf16 matmul psum += lhsT.T @ rhs, moving free dim split to <=512."""
    n = rhs.shape[-1]
    for o in range(0, n, nmax):
        w = min(nmax, n - o)
        nc.tensor.matmul(
            psum[:, o : o + w], lhsT, rhs[:, o : o + w], start=first, stop=last
        )


def mm_dbl(nc, psum, w_s, mcol, x_s, ncols, nmax=512):
    """fp8 DoubleRow projection: psum[:, :ncols] = sum_kc
    w_s[:, kc, mcol:mcol+128].T @ x_s[:, kc, :ncols], kc paired 2-at-a-time."""
    for kp in range(4):
        first, last = kp == 0, kp == 3
        for o in range(0, ncols, nmax):
            w = min(nmax, ncols - o)
            nc.tensor.matmul(
                psum[:, o : o + w],
                w_s[:, 2 * kp : 2 * kp + 2, mcol : mcol + P],
                x_s[:, 2 * kp : 2 * kp + 2, o : o + w],
                start=first, stop=last, perf_mode=DBL,
            )


def build(flags):
    """flags: (use_mask, use_npm, use_g1, use_be1, use_g2, use_be2, use_b2)"""
    use_mask, use_npm, use_g1, use_be1, use_g2, use_be2, use_b2 = flags
    nc = bacc.Bacc(None, target_bir_lowering=False)

    # ---------------- I/O ----------------
    sb_t = nc.dram_tensor("sb", [1, 1], U32, kind="ExternalInput")
    xqT = nc.dram_tensor("xqT", [D, XQW], FP8, kind="ExternalInput")
    kvT = nc.dram_tensor("kvT", [D, KL], FP8, kind="ExternalInput")
    peT = nc.dram_tensor("peT", [D, KL], FP8, kind="ExternalInput")
    xres = nc.dram_tensor("xres", [QL, D], F32, kind="ExternalInput")
    Wq = nc.dram_tensor("Wq", [D, HK], FP8, kind="ExternalInput")
    Wk = nc.dram_tensor("Wk", [D, HK], FP8, kind="ExternalInput")
    Wv = nc.dram_tensor("Wv", [D, HK], FP8, kind="ExternalInput")
    Wr = nc.dram_tensor("Wr", [D, HK], FP8, kind="ExternalInput")
    Wo = nc.dram_tensor("Wo", [HK, D], BF16, kind="ExternalInput")
    W1 = nc.dram_tensor("W1", [D, F], BF16, kind="ExternalInput")
    W2 = nc.dram_tensor("W2", [F, D], BF16, kind="ExternalInput")
    cbt = nc.dram_tensor("cbt", [P, HK // P], F32, kind="ExternalInput")
    pbt = nc.dram_tensor("pbt", [P, HK // P], F32, kind="ExternalInput")
    b1t = nc.dram_tensor("b1t", [P, F // P], F32, kind="ExternalInput")
    if use_mask:
        maskadd = nc.dram_tensor("maskadd", [KL, QL], F32, kind="ExternalInput")
    if use_npm:
        npmt = nc.dram_tensor("npmt", [P, QL // P], F32, kind="ExternalInput")
    row_vecs = {}
    for name, used in (
        ("g1r", use_g1), ("be1r", use_be1), ("g2r", use_g2),
        ("be2r", use_be2), ("b2r", use_b2),
    ):
        if used:
            row_vecs[name] = nc.dram_tensor(name, [1, D], F32, kind="ExternalInput")
    out_t = nc.dram_tensor("out", [QL, D], F32, kind="ExternalOutput")

    def bcast_row(t):
        return bass.AP(tensor=t.tensor, offset=t.offset, ap=[[0, P], [1, D]])

    DS = 1.0 / WSCALE

    with tile.TileContext(nc) as tc:
        with (
            tc.tile_pool(name="const", bufs=1) as const,
            tc.tile_pool(name="acts", bufs=1) as acts,
            tc.tile_pool(name="ydram", bufs=1, space="DRAM") as ydram,
        ):
            # =============== PHASE A: DMA triggers first, ring-spread ===============
            # rings (only sync/scalar/gpsimd can DMA):
            #   sync:   xqT, Wq[4:8], Wk[4:8], peT
            #   scalar: Wq[0:4], Wk[0:4], Wv
            #   gpsimd: kvT, Wr, consts
            abuf_ctx = tc.tile_pool(name="abuf", bufs=1)
            abuf = abuf_ctx.__enter__()
            pa1_ctx = tc.tile_pool(name="pa_x1", bufs=1)
            pa_x = pa1_ctx.__enter__()

            xqT_s = pa_x.tile([P, 8, XQW], FP8, name="xqT_s", tag="xqT_s")
            nc.sync.dma_start(out=xqT_s, in_=xqT.rearrange("(c p) k -> p c k", p=P))
            wq_s = pa_x.tile([P, 8, HK], FP8, name="wq_s", tag="wq_s")
            wq_r = Wq.rearrange("(c p) n -> p c n", p=P)
            nc.scalar.dma_start(out=wq_s[:, 0:4, :], in_=wq_r[:, 0:4, :])
            nc.sync.dma_start(out=wq_s[:, 4:8, :], in_=wq_r[:, 4:8, :])
            kvT_s = abuf.tile([P, 8, KL], FP8, name="kvT_s", tag="kvT_s")
            kvT_r = kvT.rearrange("(c p) k -> p c k", p=P)
            nc.gpsimd.dma_start(out=kvT_s, in_=kvT_r)
            wk_s = pa_x.tile([P, 8, HK], FP8, name="wk_s", tag="wk_s")
            wk_r = Wk.rearrange("(c p) n -> p c n", p=P)
            nc.scalar.dma_start(out=wk_s[:, 0:4, :], in_=wk_r[:, 0:4, :])
            nc.sync.dma_start(out=wk_s[:, 4:8, :], in_=wk_r[:, 4:8, :])
            peT_s = pa_x.tile([P, 8, KL], FP8, name="peT_s", tag="peT_s")
            nc.sync.dma_start(out=peT_s, in_=peT.rearrange("(c p) k -> p c k", p=P))
            wr_s = pa_x.tile([P, 8, HK], FP8, name="wr_s", tag="wr_s")
            nc.gpsimd.dma_start(out=wr_s, in_=Wr.rearrange("(c p) n -> p c n", p=P))
            wv_s = abuf.tile([P, 8, HK], FP8, name="wv_s", tag="wv_s")
            nc.scalar.dma_start(out=wv_s, in_=Wv.rearrange("(c p) n -> p c n", p=P))

            # small consts (gpsimd ring, after the big loads)
            cb_s = const.tile([P, HK // P], F32)
            pb_s = const.tile([P, HK // P], F32)
            b1_s = const.tile([P, F // P], F32)
            nc.gpsimd.dma_start(out=cb_s, in_=cbt[:])
            nc.gpsimd.dma_start(out=pb_s, in_=pbt[:])
            nc.gpsimd.dma_start(out=b1_s, in_=b1t[:])
            if use_npm:
                npm_s = const.tile([P, QL // P], F32)
                nc.gpsimd.dma_start(out=npm_s, in_=npmt[:])
            rv = {}
            for name in row_vecs:
                rv[name] = const.tile([P, D], F32, name=f"rv_{name}", tag=f"rv_{name}")
                nc.gpsimd.dma_start(out=rv[name], in_=bcast_row(row_vecs[name]))
            if use_mask:
                mask_s = const.tile([P, 8, QL], F32)
                nc.gpsimd.dma_start(
                    out=mask_s, in_=maskadd.rearrange("(c p) a -> p c a", p=P)
                )
            xres_s = const.tile([P, 2, D], F32)
            nc.gpsimd.dma_start(
                out=xres_s, in_=xres.rearrange("(t p) d -> p t d", p=P)
            )

            # consts: garb on vector (free at t0) so warm-up can start immediately
            garb = const.tile([P, 512], BF16)
            nc.vector.memset(garb, 0.125)
            ident = const.tile([P, P], BF16)
            make_identity(nc, ident)
            eps_t = const.tile([P, 1], F32)
            nc.vector.memset(eps_t, EPS)

            # persistent activation tensors
            kT_t = [abuf.tile([P, KL], BF16, name=f"kT{i}", tag=f"kT{i}")
                    for i in range(8)]
            rT_t = [abuf.tile([P, KL], BF16, name=f"rT{i}", tag=f"rT{i}")
                    for i in range(8)]
            v_s = abuf.tile([P, 8, H * VW], BF16)   # [klchunk rows, chunk, head*65]
            qT_t = [abuf.tile([P, YROWS], BF16, name=f"qT{i}", tag=f"qT{i}")
                    for i in range(8)]
            ctxT_s = acts.tile([P, 8, QL], BF16)
            y_s = acts.tile([P, 2, D], F32)
            yT_s = acts.tile([P, 8, QL], BF16)

            # ones columns of v (col 64 of each 65-wide head block)
            nc.gpsimd.memset(
                v_s.rearrange("p m (h c) -> p m h c", c=VW)[:, :, :, DH : DH + 1],
                1.0,
            )

            # per-core shift register for the rel_shift reads (loaded during phase A,
            # used by the phase-B transpose DMAs on sync/scalar)
            regs = nc.alloc_registers(
                "sbase", engines=[mybir.EngineType.SP, mybir.EngineType.Activation]
            )
            nc.regs_load(regs, sb_t[0:1, 0:1])
            sb_sv = nc.snap(regs, donate=True, min_val=0, max_val=512)

            # one big DRAM y buffer; per-head flat/2D views
            YTILE = (YROWS + 1) * YW
            ybig = ydram.tile([H * YTILE], BF16, name="ybig", tag="ybig")
            y1_t = [ybig[h * YTILE : (h + 1) * YTILE] for h in range(H)]
            y2d_t = [y1.rearrange("(a b) -> a b", b=YW) for y1 in y1_t]
            # halo-row view grouped (p, j): head h = 2p+j row QL, all YW cols
            yb_pj = ybig.rearrange("(p j ab) -> p j ab", j=2, ab=YTILE)

            halo_wr = [[] for _ in range(H)]

            # B-phase pools open mid-A (after the q/k/r fp8 inputs free); the
            # first three attention fronts are emitted inside phase A so the v
            # projection's dense matmul stream covers their y->DRAM->bdT trips
            w4 = w2p = ps_sc = ps_sa = ps_cx = None  # bound before first use
            qcT_l = [None] * H
            bdT_l = [None] * H
            pT_l = [None] * H
            tr_l = [None] * H
            ad_hist = [[] for _ in range(H)]

            def stage_front(h):
                hi, hr = h // 2, (h % 2) * DH
                qT_h = qT_t[hi][hr : hr + DH, :]
                rT_h = rT_t[hi][hr : hr + DH, :]
                qcT_f = w4.tile([P, QL], BF16, name="qcT", tag="qcT")
                nc.gpsimd.tensor_scalar(
                    out=qcT_f[hr : hr + DH, :], in0=qT_h[:, :QL],
                    scalar1=cb_s[hr : hr + DH, hi : hi + 1], scalar2=0.125,
                    op0=mybir.AluOpType.add, op1=mybir.AluOpType.mult,
                )
                qcT_l[h] = qcT_f
                qpT_f = w4.tile([P, QL], BF16, name="qpT", tag="qpT")
                qpT = qpT_f[hr : hr + DH, :]
                nc.gpsimd.tensor_scalar(
                    out=qpT, in0=qT_h[:, :QL],
                    scalar1=pb_s[hr : hr + DH, hi : hi + 1], scalar2=0.125,
                    op0=mybir.AluOpType.add, op1=mybir.AluOpType.mult,
                )
                wr_insts = list(halo_wr[h])
                bw = w2p.tile([P, 2, YW], BF16, name="bw", tag="bw")
                nc.gpsimd.memset(bw[:, :, 0:1], 0.0)
                for t in range(2):
                    for o in range(2):
                        psb = ps_sc.tile([P, 512], F32, name="psb", tag="sc")
                        nc.tensor.matmul(
                            psb, qpT[:, t * P : (t + 1) * P],
                            rT_h[:, o * 512 : (o + 1) * 512],
                            start=True, stop=True,
                        )
                        if t == 0:
                            nc.scalar.copy(
                                out=bw[:, t, 1 + o * 512 : 513 + o * 512],
                                in_=psb,
                            )
                        else:
                            nc.vector.tensor_copy(
                                bw[:, t, 1 + o * 512 : 513 + o * 512], psb
                            )
                # y write on sync: keeps gpsimd's queue free for the next
                # head's qc/qp prep, and orders naturally before our tr
                wr_insts.append(
                    nc.sync.dma_start(
                        out=y2d_t[h][0:QL, :].rearrange(
                            "(t p) w -> p t w", p=P
                        ),
                        in_=bw,
                    )
                )
                bdT = w4.tile([P, 8, QL], BF16, name="bdT", tag="bdT")
                bdT_l[h] = bdT
                tr = nc.sync.dma_start(
                    out=bdT,
                    in_=y1_t[h][bass.ds(sb_sv, QL * KL)].rearrange(
                        "(a b) -> a b", b=KL
                    ),
                    transpose=True,
                )
                tr_l[h] = tr
                for wi in wr_insts:
                    add_dep_helper(tr.ins, wi.ins, reason="bdT read waits y writes")
                if h >= 4:
                    for prev_ad in ad_hist[h - 4]:
                        add_dep_helper(
                            tr.ins, prev_ad.ins,
                            reason="bdT slot reuse waits prior readers",
                        )

            def stage_mid(h):
                hi, hr = h // 2, (h % 2) * DH
                kT_h = kT_t[hi][hr : hr + DH, :]
                qcT = qcT_l[h][hr : hr + DH, :]
                pT = w4.tile([P, 8, QL], BF16, name="pT", tag="pT")
                pT_l[h] = pT
                # quarter-head psum groups (1 bank each, 4 slots): S^T chunk
                # = identity-matmul of bdT (start=True fills the 2KB
                # zero-region) + 2 AC accumulations, exp'd immediately so
                # the slot frees fast and heads overlap
                for g in range(4):
                    psa = ps_sa.tile([P, 2, QL], F32, name="psa", tag="sa")
                    ad = nc.tensor.matmul(
                        psa, ident, bdT_l[h][:, 2 * g : 2 * g + 2, :],
                        start=True, stop=False, skip_group_check=True,
                    )
                    add_dep_helper(
                        ad.ins, tr_l[h].ins, reason="bd add waits bdT"
                    )
                    ad_hist[h].append(ad)
                    for cc in (0, 1):
                        c = 2 * g + cc
                        nc.tensor.matmul(
                            psa[:, cc, :], kT_h[:, c * P : (c + 1) * P],
                            qcT, start=False, stop=(cc == 1),
                            skip_group_check=True,
                        )
                    if use_mask:
                        nc.vector.tensor_add(
                            out=psa, in0=psa,
                            in1=mask_s[:, 2 * g : 2 * g + 2, :],
                        )
                    nc.scalar.activation(
                        out=pT[:, 2 * g : 2 * g + 2, :], in_=psa,
                        func=mybir.ActivationFunctionType.Exp,
                    )

            def stage_back(h):
                hi, hr = h // 2, (h % 2) * DH
                pT = pT_l[h]
                psc = ps_cx.tile([VW, QL], F32, name="psc", tag="cx")
                for j in range(8):
                    nc.tensor.matmul(
                        psc, v_s[:, j, h * VW : (h + 1) * VW], pT[:, j, :],
                        start=(j == 0), stop=(j == 7),
                    )
                srow = w2p.tile([1, QL], F32, name="srow", tag="srow")
                nc.vector.tensor_copy(srow, psc[DH : DH + 1, :])
                rrow = w2p.tile([1, QL], F32, name="rrow", tag="rrow")
                nc.vector.reciprocal_approx_fast(out=rrow, in_=srow)
                rcb = w2p.tile([DH, QL], F32, name="rcb", tag="rcb")
                nc.gpsimd.partition_broadcast(rcb, rrow)
                nc.vector.tensor_mul(
                    out=ctxT_s[hr : hr + DH, hi, :], in0=psc[0:DH, :], in1=rcb
                )

            paps1_ctx = tc.tile_pool(name="pa_ps1", bufs=3, space="PSUM")
            pa_ps = paps1_ctx.__enter__()
            if True:
                # HAM warm-up: keep PE busy from t~1us so real matmuls run at 2.4GHz
                wps = pa_ps.tile([P, 512], F32, name="warm", tag="pa_psum")
                for _ in range(48):
                    nc.tensor.matmul(wps, garb[:, 0:P], garb, start=True, stop=True)

                # ---- q projection (fp8 DoubleRow) ----
                for m in range(8):
                    ps = pa_ps.tile([P, XQW], F32, name=f"ps_q{m}", tag="pa_psum")
                    mm_dbl(nc, ps, wq_s, m * P, xqT_s, XQW)
                    nc.scalar.activation(
                        out=qT_t[m], in_=ps[:, :YROWS],
                        func=mybir.ActivationFunctionType.Copy, scale=DS,
                    )
                # ---- k projection ----
                for m in range(8):
                    ps = pa_ps.tile([P, KL], F32, name=f"ps_k{m}", tag="pa_psum")
                    mm_dbl(nc, ps, wk_s, m * P, kvT_s, KL)
                    nc.vector.tensor_scalar_mul(out=kT_t[m], in0=ps, scalar1=DS)
                # ---- r projection ----
                for m in range(8):
                    ps = pa_ps.tile([P, KL], F32, name=f"ps_r{m}", tag="pa_psum")
                    mm_dbl(nc, ps, wr_s, m * P, peT_s, KL)
                    nc.scalar.activation(
                        out=rT_t[m], in_=ps,
                        func=mybir.ActivationFunctionType.Copy, scale=DS,
                    )
                # ---- halo BD rows (pair-batched, after A-psum frees) ----
                paps1_ctx.__exit__(None, None, None)
                pa1_ctx.__exit__(None, None, None)
                with (
                    tc.tile_pool(name="ph_w", bufs=2) as ph_w,
                    tc.tile_pool(name="ph_ps", bufs=3, space="PSUM") as ph_ps,
                ):
                    qph = ph_w.tile([P, H], BF16, name="qph", tag="qph")
                    nc.vector.memset(qph, 0.0)
                    for h in range(H):
                        hi, hr = h // 2, (h % 2) * DH
                        nc.vector.tensor_scalar(
                            out=qph[hr : hr + DH, h : h + 1],
                            in0=qT_t[hi][hr : hr + DH, QL : QL + 1],
                            scalar1=pb_s[hr : hr + DH, hi : hi + 1], scalar2=0.125,
                            op0=mybir.AluOpType.add, op1=mybir.AluOpType.mult,
                        )
                    hbf = ph_w.tile([2, 8, YW], BF16, name="hbf", tag="hbf")
                    nc.gpsimd.memset(hbf[:, :, 0:1], 0.0)
                    for p in range(8):
                        psh = ph_ps.tile([2, KL], F32, name=f"psh{p}", tag="psh")
                        for o in (0, 512):
                            nc.tensor.matmul(
                                psh[:, o : o + 512], qph[:, 2 * p : 2 * p + 2],
                                rT_t[p][:, o : o + 512], start=True, stop=True,
                            )
                        if p % 2 == 0:
                            nc.scalar.copy(out=hbf[:, p, 1:], in_=psh)
                        else:
                            nc.vector.tensor_copy(hbf[:, p, 1:], psh)
                    for j in (0, 1):
                        wj = nc.sync.dma_start(
                            out=yb_pj[:, j, QL * YW : (QL + 1) * YW],
                            in_=hbf[j : j + 1, :, :],
                        )
                        for p in range(8):
                            halo_wr[2 * p + j] = [wj]

                # first attention fronts, covered by the v projection
                b4_ctx = tc.tile_pool(name="pb_w4", bufs=4)
                w4 = b4_ctx.__enter__()
                b2_ctx = tc.tile_pool(name="pb_w2", bufs=3)
                w2p = b2_ctx.__enter__()
                sc_ctx = tc.tile_pool(name="pb_ps_sc", bufs=2, space="PSUM")
                ps_sc = sc_ctx.__enter__()
                wm_ctx = tc.tile_pool(name="pb_ps_wm", bufs=1, space="PSUM")
                ps_wm = wm_ctx.__enter__()
                paps2_ctx = tc.tile_pool(name="pa_ps2", bufs=2, space="PSUM")
                pa_ps = paps2_ctx.__enter__()
                for h0 in range(3):
                    stage_front(h0)
                    for _ in range(6):
                        nc.tensor.ldweights(garb[:, 0:P])

                # ---- v projection: lhsT = kvT chunks, rhs = Wv ----
                v_view = v_s.rearrange("p m (h c) -> p m h c", c=VW)
                for m in range(8):  # kl-chunks
                    ps = pa_ps.tile([P, HK], F32, name=f"ps_v{m}", tag="pa_psum")
                    mm_dbl(nc, ps, kvT_s, m * P, wv_s, HK)
                    nc.vector.tensor_scalar_mul(
                        out=v_view[:, m, :, 0:DH],
                        in0=ps.rearrange("p (h c) -> p h c", c=DH),
                        scalar1=DS,
                    )

            paps2_ctx.__exit__(None, None, None)

            # =================== PHASE B: pipelined attention ===================
            sa_ctx = tc.tile_pool(name="pb_ps_sa", bufs=3, space="PSUM")
            ps_sa = sa_ctx.__enter__()
            cx_ctx = tc.tile_pool(name="pb_ps_cx", bufs=2, space="PSUM")
            ps_cx = cx_ctx.__enter__()

            # prefetch Wo + first W1 chunks during attention
            wo_s = acts.tile([P, 8, D], BF16, name="wo_s", tag="wo_s")
            wo_r = Wo.rearrange("(c p) n -> p c n", p=P)
            nc.scalar.dma_start(out=wo_s[:, 0:4, :], in_=wo_r[:, 0:4, :])
            nc.gpsimd.dma_start(out=wo_s[:, 4:8, :], in_=wo_r[:, 4:8, :])
            w1a_s = acts.tile([P, 8, 1024], BF16, name="w1a_s", tag="w1a_s")
            w1_r = W1.rearrange("(c p) n -> p c n", p=P)
            nc.scalar.dma_start(out=w1a_s[:, :, 0:512], in_=w1_r[:, :, 0:512])
            nc.gpsimd.dma_start(
                out=w1a_s[:, :, 512:1024], in_=w1_r[:, :, 512:1024]
            )

            for s in range(2, H + 3):
                if 3 <= s < H:
                    stage_front(s)
                # HAM insurance: dep-free filler matmuls plug sub-us holes in
                # the PE stream so the clock gate stays at 2.4GHz
                for _ in range(6 if s < 6 else 2):
                    wfil = ps_wm.tile([P, 512], F32, name="wfil", tag="wfil")
                    nc.tensor.matmul(wfil, garb[:, 0:P], garb, start=True, stop=True)
                if 0 <= s - 2 < H:
                    stage_mid(s - 2)
                if 0 <= s - 3 < H:
                    stage_back(s - 3)

            cx_ctx.__exit__(None, None, None)
            sa_ctx.__exit__(None, None, None)
            wm_ctx.__exit__(None, None, None)
            sc_ctx.__exit__(None, None, None)
            b2_ctx.__exit__(None, None, None)
            b4_ctx.__exit__(None, None, None)
            abuf_ctx.__exit__(None, None, None)

            # =================== PHASE C: Wo + LN1 ===================
            with (
                tc.tile_pool(name="pc_work", bufs=2) as cwork,
                tc.tile_pool(name="pc_ps", bufs=2, space="PSUM") as pc_ps,
                tc.tile_pool(name="pc_ps_tp", bufs=2, space="PSUM") as pc_ps_tp,
            ):

                def layer_norm(dst, u, gname, bname):
                    stats = cwork.tile([P, 2, 6], F32, name="stats", tag="stats")
                    for sg in range(2):
                        nc.vector.bn_stats(
                            out=stats[:, sg, :], in_=u[:, sg * 512 : (sg + 1) * 512]
                        )
                    mv = cwork.tile([P, 2], F32, name="mv", tag="mv")
                    nc.vector.bn_aggr(out=mv, in_=stats)
                    rstd = cwork.tile([P, 1], F32, name="rstd", tag="rstd")
                    nc.scalar.activation(
                        out=rstd, in_=mv[:, 1:2],
                        func=mybir.ActivationFunctionType.Sqrt, bias=eps_t,
                    )
                    nc.vector.reciprocal(out=rstd, in_=rstd)
                    nc.vector.tensor_scalar(
                        out=dst, in0=u, scalar1=mv[:, 0:1], scalar2=rstd,
                        op0=mybir.AluOpType.subtract, op1=mybir.AluOpType.mult,
                    )
                    if gname:
                        nc.vector.tensor_mul(out=dst, in0=dst, in1=rv[gname])
                    if bname:
                        nc.vector.tensor_add(out=dst, in0=dst, in1=rv[bname])

                pso_t, ybf_t = [], []
                for t in range(2):
                    pso = pc_ps.tile([P, D], F32, name=f"pso{t}", tag="pso")
                    pso_t.append(pso)
                    for j in range(8):
                        mm_acc(nc, pso, ctxT_s[:, j, t * P : (t + 1) * P],
                               wo_s[:, j, :], first=(j == 0), last=(j == 7))
                    u1 = cwork.tile([P, D], F32, name="u1", tag="u1")
                    nc.vector.tensor_add(out=u1, in0=pso, in1=xres_s[:, t, :])
                    layer_norm(
                        y_s[:, t, :], u1,
                        "g1r" if use_g1 else None, "be1r" if use_be1 else None,
                    )
                    if use_npm:
                        nc.vector.tensor_scalar_mul(
                            out=y_s[:, t, :], in0=y_s[:, t, :],
                            scalar1=npm_s[:, t : t + 1],
                        )
                    ybf = cwork.tile([P, D], BF16, name="ybf", tag="ybf")
                    nc.scalar.copy(out=ybf, in_=y_s[:, t, :])
                    ybf_t.append(ybf)
                for t in range(2):
                    for j in range(8):
                        tp = pc_ps_tp.tile([P, P], BF16, name="tp2", tag="tp2")
                        nc.tensor.transpose(tp, ybf_t[t][:, j * P : (j + 1) * P], ident)
                        nc.scalar.copy(out=yT_s[:, j, t * P : (t + 1) * P], in_=tp)

                # =================== PHASE D: MLP + LN2 ===================
                with (
                    tc.tile_pool(name="pd_w", bufs=2) as pd_w,
                    tc.tile_pool(name="pd_w2", bufs=1) as pd_w2,
                    tc.tile_pool(name="pd_h", bufs=1) as pd_h,
                    tc.tile_pool(name="pd_ps_h", bufs=2, space="PSUM") as pd_ps_h,
                ):
                    h1T_g = [
                        pd_h.tile([P, 8, QL], BF16, name=f"h1T{g}", tag=f"h1T{g}")
                        for g in range(4)
                    ]
                    for jb in range(8):  # batches of 4 f-chunks (512 cols of W1)
                        if jb < 2:
                            w1 = w1a_s[:, :, jb * 512 : (jb + 1) * 512]
                        else:
                            w1t = pd_w.tile([P, 8, 512], BF16, name="w1", tag="w1")
                            (nc.scalar, nc.gpsimd)[jb % 2].dma_start(
                                out=w1t, in_=w1_r[:, :, jb * 512 : (jb + 1) * 512]
                            )
                            w1 = w1t
                        for jj in range(4):
                            j = jb * 4 + jj
                            psh1 = pd_ps_h.tile([P, QL], F32, name="psh1", tag="psh1")
                            for kc in range(8):
                                nc.tensor.matmul(
                                    psh1, w1[:, kc, jj * P : (jj + 1) * P],
                                    yT_s[:, kc, :],
                                    start=(kc == 0), stop=(kc == 7),
                                )
                            nc.scalar.activation(
                                out=h1T_g[j // 8][:, j % 8, :], in_=psh1,
                                func=mybir.ActivationFunctionType.Relu,
                                bias=b1_s[:, j : j + 1],
                            )
                    w2_s = [None] * 8
                    for jb in range(8):
                        w2t = pd_w2.tile([P, 4, D], BF16, name="w2", tag=f"w2_{jb}")
                        (nc.gpsimd, nc.scalar)[jb % 2].dma_start(
                            out=w2t,
                            in_=W2.rearrange("(c p) n -> p c n", p=P)[
                                :, jb * 4 : (jb + 1) * 4, :
                            ],
                        )
                        w2_s[jb] = w2t
                    for t in range(2):  # t-outer: LN2+store of t=0 overlaps t=1 mms
                        ps2 = pc_ps.tile([P, D], F32, name=f"ps2_{t}", tag="pso")
                        for j in range(F // P):
                            mm_acc(nc, ps2,
                                   h1T_g[j // 8][:, j % 8, t * P : (t + 1) * P],
                                   w2_s[j // 4][:, j % 4, :],
                                   first=(j == 0), last=(j == F // P - 1))
                        u2 = cwork.tile([P, D], F32, name="u2", tag="u1")
                        nc.vector.tensor_add(out=u2, in0=ps2, in1=y_s[:, t, :])
                        if use_b2:
                            nc.vector.tensor_add(out=u2, in0=u2, in1=rv["b2r"])
                        o2 = cwork.tile([P, D], F32, name="o2", tag="o2")
                        layer_norm(
                            o2, u2,
                            "g2r" if use_g2 else None, "be2r" if use_be2 else None,
                        )
                        if use_npm:
                            nc.vector.tensor_scalar_mul(
                                out=o2, in0=o2, scalar1=npm_s[:, t : t + 1]
                            )
                        nc.gpsimd.dma_start(
                            out=out_t.rearrange("(t p) d -> p t d", p=P)[:, t, :],
                            in_=o2,
                        )

    nc.compile()
    return nc


def _host_prep(inputs):
    """Shared (core-independent) host prep: fp8/bf16 casts + pe table."""
    f32 = lambda x: np.asarray(x, np.float32)
    bf = lambda x: np.asarray(x, np.float32).astype(NP_BF16)
    f8w = lambda x: (np.asarray(x, np.float32) * WSCALE).astype(NP_FP8)

    pos = np.arange(KL - 1, -1, -1, dtype=np.float32)
    inv = (1.0 / (10000.0 ** (np.arange(0, D, 2, dtype=np.float32) / D))).astype(
        np.float32
    )
    ang = pos[:, None] * inv
    pe = np.concatenate([np.sin(ang), np.cos(ang)], axis=-1).astype(np.float32)

    shared = {
        "peT": np.ascontiguousarray(pe.T).astype(NP_FP8),
        "Wq": f8w(inputs["Wq"]), "Wk": f8w(inputs["Wk"]),
        "Wv": f8w(inputs["Wv"]), "Wr": f8w(inputs["Wr"]),
        "Wo": bf(inputs["Wo"]), "W1": bf(inputs["W1"]), "W2": bf(inputs["W2"]),
        "cbt": f32(inputs["content_bias"]).reshape(HK).reshape(8, P).T.copy(),
        "pbt": f32(inputs["position_bias"]).reshape(HK).reshape(8, P).T.copy(),
        "b1t": f32(inputs["b1"]).reshape(F // P, P).T.copy(),
    }
    return shared


def _run(inputs, trace=False):
    x = np.asarray(inputs["layer_input"], np.float32)
    mem = np.asarray(inputs["memory"], np.float32)
    npm = np.asarray(inputs["non_pad_mask"], np.float32)
    mask = np.asarray(inputs["slf_attn_mask"])
    g1 = np.asarray(inputs["ln1_g"], np.float32)
    be1 = np.asarray(inputs["ln1_b"], np.float32)
    g2 = np.asarray(inputs["ln2_g"], np.float32)
    be2 = np.asarray(inputs["ln2_b"], np.float32)
    b2 = np.asarray(inputs["b2"], np.float32)

    flags = (
        bool(mask.any()),
        not bool(np.all(npm == 1.0)),
        not bool(np.all(g1 == 1.0)),
        bool(be1.any()),
        not bool(np.all(g2 == 1.0)),
        bool(be2.any()),
        bool(b2.any()),
    )
    if flags not in _cache:
        _cache[flags] = build(flags)
    nc = _cache[flags]
    use_mask, use_npm, use_g1, use_be1, use_g2, use_be2, use_b2 = flags

    shared = _host_prep(inputs)
    in_maps = []
    for c in range(NCORES):
        b, qh = c // 2, c % 2
        lo = QL * qh
        kv = np.concatenate([mem[b], x[b]], axis=0)  # [KL, D]
        xq = np.zeros((XQW, D), np.float32)
        hi = min(lo + YROWS, Q)
        xq[: hi - lo] = x[b, lo:hi]
        m = dict(shared)
        m["sb"] = np.array([[512 - lo]], np.uint32)
        m["xqT"] = np.ascontiguousarray(xq.T).astype(NP_FP8)
        m["kvT"] = np.ascontiguousarray(kv.T).astype(NP_FP8)
        m["xres"] = np.ascontiguousarray(x[b, lo : lo + QL])
        if use_mask:
            m["maskadd"] = np.where(
                mask[b, lo : lo + QL], np.float32(-1e9), np.float32(0)
            ).astype(np.float32)
        if use_npm:
            m["npmt"] = npm[b, lo : lo + QL, 0].reshape(2, P).T.copy()
        if use_g1:
            m["g1r"] = g1.reshape(1, D).copy()
        if use_be1:
            m["be1r"] = be1.reshape(1, D).copy()
        if use_g2:
            m["g2r"] = g2.reshape(1, D).copy()
        if use_be2:
            m["be2r"] = be2.reshape(1, D).copy()
        if use_b2:
            m["b2r"] = b2.reshape(1, D).copy()
        in_maps.append(m)

    res = run_bass_kernel_spmd(nc, in_maps, core_ids=list(range(NCORES)), trace=trace)
    out = np.empty((B, Q, D), np.float32)
    for c in range(NCORES):
        b, qh = c // 2, c % 2
        out[b, QL * qh : QL * (qh + 1)] = res.results[c]["out"]
    return out, res


def kernel(**inputs):
    out, _ = _run(inputs, trace=False)
    return out
